# revision 18
# baseline (speedup 1.0000x reference)
"""Trainium2 Bass kernel for nn_MultiHeadHyperNet.

Strategy (8 NeuronCores, SPMD, 3 launches):
  L1: column-sum of X_train shards (data-parallel over rows) -> host mean+encoder.
  L2: hypernet head matvec. Only 467 of 983 params/tree are ever used
      (split_w i<3, split_b i<3, leaf logits), so only those rows of head_W2
      are read (143MB instead of 302MB), host-transposed to [H, rows] and
      host-cast to bf16, sharded by rows across cores. Stationary = hh^T
      chunks [128, 10] (all classes at once); moving = W2^T tiles. The
      correct class row per column is selected on host afterwards.
  L3: routing, data-parallel over X_test rows. Per (class,tree) the leaf
      mixture is an order-3 multilinear polynomial in the 3 routing
      sigmoids, so out[b,k] = sum over 1050 monomial features x A[f,k]:
      features = 450 routes + 450 pair products + 150 triple products.
      Routes via matmul (stationary split_w^T), sigmoid on ACT (per-
      partition bias), products on DVE in bf16, final [K~1050]x[10] matmul
      accumulated in PSUM. The (c,t) axis is split in two groups of 75
      (padded to 128 partitions).

All matmul inputs bf16 (fp32 PSUM accumulation); validated end-to-end
rel err ~2e-4 vs the fp32 reference.
"""
import numpy as np
import ml_dtypes

import concourse.bacc as bacc
import concourse.mybir as mybir
import concourse.tile as tile
from concourse.bass_utils import run_bass_kernel_spmd

BF16 = mybir.dt.bfloat16
F32 = mybir.dt.float32
BFNP = ml_dtypes.bfloat16

NCORES = 8
D, H, C, T, DEPTH = 128, 512, 10, 15, 3
I, L = 2 ** DEPTH - 1, 2 ** DEPTH
PPT = I * (D + 1) + L * C        # 983
NCT = C * T                      # 150
USED = 3 * D + 3 + L * C         # 467 used params per (c,t)
RPT = T * USED                   # 7005 used rows per class
RTOT = C * RPT                   # 70050 used rows total
LN_EPS = 1e-5

# L1 / L3 batch sharding
BTR_CORE = 100000 // NCORES      # 12500
BT = 512
NBT = 25                         # b-tiles per core (12800 padded)
BPAD = NBT * BT

# L2 column sharding
L2_TILES = 18                    # 18*512 = 9216 cols per core
L2_COLS = L2_TILES * BT          # 9216 ;  8*9216 = 73728 >= 70050

# L3 ct grouping: 2 groups of 75 (padded to 128 partitions)
GSZ = 75
N_RCHUNK = 6                     # (g, d) route chunks
N_PCHUNK = 8                     # (g, {01,02,12,012}) product chunks
N_FCHUNK = N_RCHUNK + N_PCHUNK   # 14 feature chunks for the final matmul

L3_VERSION = 3

USED_OFF = np.concatenate([
    np.arange(3 * D),              # split_w i<3
    I * D + np.arange(3),          # split_b i<3
    I * D + I + np.arange(L * C),  # leaf logits
]).astype(np.int64)

_CACHE = {}


# ----------------------------------------------------------------- kernels
def _build_l1():
    nc = bacc.Bacc("TRN2", target_bir_lowering=False, debug=False,
                   num_devices=NCORES)
    xt = nc.dram_tensor("xt", [128, BTR_CORE], BF16, kind="ExternalInput")
    s = nc.dram_tensor("s", [128, 1], F32, kind="ExternalOutput")
    with tile.TileContext(nc) as tc:
        with tc.tile_pool(name="sb", bufs=2) as sb:
            CH = 2500
            acc = sb.tile([128, BTR_CORE // CH], F32)
            for j in range(BTR_CORE // CH):
                t = sb.tile([128, CH], BF16, tag="xt")
                nc.sync.dma_start(t[:], xt[:, j * CH:(j + 1) * CH])
                nc.vector.reduce_sum(acc[:, j:j + 1], t[:],
                                     axis=mybir.AxisListType.X)
            out = sb.tile([128, 1], F32)
            nc.vector.reduce_sum(out[:], acc[:], axis=mybir.AxisListType.X)
            nc.sync.dma_start(s[:], out[:])
    nc.compile()
    return nc


def _build_l2():
    nc = bacc.Bacc("TRN2", target_bir_lowering=False, debug=False,
                   num_devices=NCORES)
    w2t = nc.dram_tensor("w2t", [H, L2_COLS], BF16, kind="ExternalInput")
    hht = nc.dram_tensor("hht", [4, 128, C], BF16, kind="ExternalInput")
    pr = nc.dram_tensor("pr", [C, L2_COLS], F32, kind="ExternalOutput")
    NSPLIT = 9
    SEG = L2_COLS // NSPLIT
    with tile.TileContext(nc) as tc:
        with (
            tc.tile_pool(name="cst", bufs=1) as cst,
            tc.tile_pool(name="mv", bufs=12) as mv,
            tc.tile_pool(name="stage", bufs=1) as stage,
            tc.tile_pool(name="ps", bufs=4, space="PSUM") as ps,
        ):
            hh_sb = cst.tile([128, 4 * C], BF16)
            for k in range(4):
                nc.sync.dma_start(hh_sb[:, k * C:(k + 1) * C], hht[k])
            out_sb = stage.tile([C, L2_COLS], F32)
            for s in range(NSPLIT):
                ws = []
                for k in range(4):
                    w = mv.tile([128, SEG], BF16, tag="w2t")
                    nc.sync.dma_start(
                        w[:], w2t[k * 128:(k + 1) * 128,
                                  s * SEG:(s + 1) * SEG])
                    ws.append(w)
                for j in range(SEG // BT):
                    acc = ps.tile([C, BT], F32)
                    for k in range(4):
                        nc.tensor.matmul(
                            acc[:], hh_sb[:, k * C:(k + 1) * C],
                            ws[k][:, j * BT:(j + 1) * BT],
                            start=(k == 0), stop=(k == 3))
                    col = s * SEG + j * BT
                    nc.vector.tensor_copy(out_sb[:, col:col + BT], acc[:])
            nc.sync.dma_start(pr[:], out_sb[:])
    nc.compile()
    return nc


def _build_l3():
    nc = bacc.Bacc("TRN2", target_bir_lowering=False, debug=False,
                   num_devices=NCORES)
    xt = nc.dram_tensor("xt", [128, BPAD], BF16, kind="ExternalInput")
    sw = nc.dram_tensor("sw", [N_RCHUNK, 128, 128], BF16, kind="ExternalInput")
    sbias = nc.dram_tensor("sbias", [128, N_RCHUNK], F32, kind="ExternalInput")
    am = nc.dram_tensor("am", [N_FCHUNK, 128, C], BF16, kind="ExternalInput")
    out = nc.dram_tensor("out", [C, BPAD], F32, kind="ExternalOutput")
    SIG = mybir.ActivationFunctionType.Sigmoid
    with tile.TileContext(nc) as tc:
        with (
            tc.tile_pool(name="cst", bufs=1) as cst,
            tc.tile_pool(name="mv", bufs=4) as mv,
            tc.tile_pool(name="feat", bufs=3) as featp,
            tc.tile_pool(name="ob", bufs=3) as obp,
            tc.tile_pool(name="ps", bufs=4, space="PSUM") as ps,
            tc.tile_pool(name="pso", bufs=3, space="PSUM") as pso,
        ):
            sw_sb = cst.tile([128, N_RCHUNK * 128], BF16)
            for i in range(N_RCHUNK):
                nc.sync.dma_start(sw_sb[:, i * 128:(i + 1) * 128], sw[i])
            a_sb = cst.tile([128, N_FCHUNK * C], BF16)
            for i in range(N_FCHUNK):
                nc.sync.dma_start(a_sb[:, i * C:(i + 1) * C], am[i])
            sb_sb = cst.tile([128, N_RCHUNK], F32)
            nc.sync.dma_start(sb_sb[:], sbias[:])

            for j in range(NBT):
                x = mv.tile([128, BT], BF16, tag="xt")
                nc.sync.dma_start(x[:], xt[:, j * BT:(j + 1) * BT])
                feat = featp.tile([128, N_FCHUNK * BT], BF16, tag="feat")

                # routes: 6 chunks (g, d)
                for i in range(N_RCHUNK):
                    rp = ps.tile([128, BT], F32, tag="route_ps")
                    nc.tensor.matmul(rp[:], sw_sb[:, i * 128:(i + 1) * 128],
                                     x[:])
                    nc.scalar.activation(feat[:, i * BT:(i + 1) * BT], rp[:],
                                         SIG, bias=sb_sb[:, i:i + 1])

                # products: for each group g: p01, p02, p12, p012
                def fsl(i):
                    return feat[:, i * BT:(i + 1) * BT]
                for g in range(2):
                    r0, r1, r2 = fsl(3 * g), fsl(3 * g + 1), fsl(3 * g + 2)
                    b = N_RCHUNK + 4 * g
                    nc.vector.tensor_mul(fsl(b), r0, r1)
                    nc.vector.tensor_mul(fsl(b + 1), r0, r2)
                    nc.vector.tensor_mul(fsl(b + 2), r1, r2)
                    nc.vector.tensor_mul(fsl(b + 3), fsl(b), r2)

                # final contraction over the 14 feature chunks
                op = pso.tile([C, BT], F32, tag="out_ps")
                for i in range(N_FCHUNK):
                    nc.tensor.matmul(op[:], a_sb[:, i * C:(i + 1) * C],
                                     fsl(i), start=(i == 0),
                                     stop=(i == N_FCHUNK - 1))
                ob = obp.tile([C, BT], F32, tag="ob")
                nc.vector.tensor_copy(ob[:], op[:])
                nc.sync.dma_start(out[:, j * BT:(j + 1) * BT], ob[:])
    nc.compile()
    return nc


def _build_l3_v3():
    """v1 layout, but the 14 final M=10 matmuls are col-tiled across 4
    32-partition col-groups of the PE array (concurrent on HW). The four
    partial strips (psum partitions 0-9/32-41/64-73/96-105) are DMA'd out
    raw and summed on host."""
    nc = bacc.Bacc("TRN2", target_bir_lowering=False, debug=False,
                   num_devices=NCORES)
    xt = nc.dram_tensor("xt", [128, BPAD], BF16, kind="ExternalInput")
    sw = nc.dram_tensor("sw", [N_RCHUNK, 128, 128], BF16, kind="ExternalInput")
    sbias = nc.dram_tensor("sbias", [128, N_RCHUNK], F32, kind="ExternalInput")
    am = nc.dram_tensor("am", [N_FCHUNK, 128, C], BF16, kind="ExternalInput")
    out = nc.dram_tensor("out", [128, BPAD], F32, kind="ExternalOutput")
    SIG = mybir.ActivationFunctionType.Sigmoid
    with tile.TileContext(nc) as tc:
        with (
            tc.tile_pool(name="cst", bufs=1) as cst,
            tc.tile_pool(name="mv", bufs=4) as mv,
            tc.tile_pool(name="feat", bufs=3) as featp,
            tc.tile_pool(name="ob", bufs=3) as obp,
            tc.tile_pool(name="ps", bufs=4, space="PSUM") as ps,
            tc.tile_pool(name="pso", bufs=3, space="PSUM") as pso,
        ):
            sw_sb = cst.tile([128, N_RCHUNK * 128], BF16)
            for i in range(N_RCHUNK):
                nc.sync.dma_start(sw_sb[:, i * 128:(i + 1) * 128], sw[i])
            a_sb = cst.tile([128, N_FCHUNK * C], BF16)
            for i in range(N_FCHUNK):
                nc.sync.dma_start(a_sb[:, i * C:(i + 1) * C], am[i])
            sb_sb = cst.tile([128, N_RCHUNK], F32)
            nc.sync.dma_start(sb_sb[:], sbias[:])

            # which final chunk is the last hitting each of the 3 col strips
            # (base partition 96 is rejected by bass AP checks, so use 3)
            last_of_strip = {}
            for i in range(N_FCHUNK):
                last_of_strip[i % 3] = i

            for j in range(NBT):
                x = mv.tile([128, BT], BF16, tag="xt")
                nc.sync.dma_start(x[:], xt[:, j * BT:(j + 1) * BT])
                feat = featp.tile([128, N_FCHUNK * BT], BF16, tag="feat")

                def fsl(i):
                    return feat[:, i * BT:(i + 1) * BT]
                for i in range(N_RCHUNK):
                    rp = ps.tile([128, BT], F32, tag="route_ps")
                    nc.tensor.matmul(rp[:], sw_sb[:, i * 128:(i + 1) * 128],
                                     x[:])
                    nc.scalar.activation(fsl(i), rp[:], SIG,
                                         bias=sb_sb[:, i:i + 1])
                for g in range(2):
                    r0, r1, r2 = fsl(3 * g), fsl(3 * g + 1), fsl(3 * g + 2)
                    b = N_RCHUNK + 4 * g
                    nc.vector.tensor_mul(fsl(b), r0, r1)
                    nc.vector.tensor_mul(fsl(b + 1), r0, r2)
                    nc.vector.tensor_mul(fsl(b + 2), r1, r2)
                    nc.vector.tensor_mul(fsl(b + 3), fsl(b), r2)

                op = pso.tile([128, BT], F32, tag="out_ps")
                for i in range(N_FCHUNK):
                    s = 32 * (i % 3)
                    nc.tensor.matmul(op[s:s + C, :],
                                     a_sb[:, i * C:(i + 1) * C], fsl(i),
                                     start=(i < 3),
                                     stop=(last_of_strip[i % 3] == i),
                                     skip_group_check=True)
                ob = obp.tile([128, BT], F32, tag="ob")
                nc.vector.tensor_copy(ob[:], op[:])
                nc.sync.dma_start(out[:, j * BT:(j + 1) * BT], ob[:])
    nc.compile()
    return nc


def _build_l3_v2():
    """(128,22) ct split: 4 route MMs + 9 final MMs per b-tile; the 22
    leftover cts' features are repacked to dense partitions via SBUF DMAs."""
    nc = bacc.Bacc("TRN2", target_bir_lowering=False, debug=False,
                   num_devices=NCORES)
    xt = nc.dram_tensor("xt", [128, BPAD], BF16, kind="ExternalInput")
    sw = nc.dram_tensor("sw", [4, 128, 128], BF16, kind="ExternalInput")
    sbias = nc.dram_tensor("sbias", [128, 4], F32, kind="ExternalInput")
    am = nc.dram_tensor("am", [9, 128, C], BF16, kind="ExternalInput")
    out = nc.dram_tensor("out", [C, BPAD], F32, kind="ExternalOutput")
    SIG = mybir.ActivationFunctionType.Sigmoid
    G2 = 22
    with tile.TileContext(nc) as tc:
        with (
            tc.tile_pool(name="cst", bufs=1) as cst,
            tc.tile_pool(name="mv", bufs=4) as mv,
            tc.tile_pool(name="feat", bufs=3) as featp,
            tc.tile_pool(name="sm", bufs=3) as smp,
            tc.tile_pool(name="ob", bufs=3) as obp,
            tc.tile_pool(name="ps", bufs=4, space="PSUM") as ps,
            tc.tile_pool(name="pso", bufs=3, space="PSUM") as pso,
        ):
            sw_sb = cst.tile([128, 4 * 128], BF16)
            for i in range(4):
                nc.sync.dma_start(sw_sb[:, i * 128:(i + 1) * 128], sw[i])
            a_sb = cst.tile([128, 9 * C], BF16)
            for i in range(9):
                nc.sync.dma_start(a_sb[:, i * C:(i + 1) * C], am[i])
            sb_sb = cst.tile([128, 4], F32)
            nc.sync.dma_start(sb_sb[:], sbias[:])

            for j in range(NBT):
                x = mv.tile([128, BT], BF16, tag="xt")
                nc.sync.dma_start(x[:], xt[:, j * BT:(j + 1) * BT])
                # g1 route chunks 0-2 + packed g2 chunk 3 -> feat[0..3]
                # feat free layout: 8 blocks of BT:
                #   0-2: R0,R1,R2(g1)  3-6: P01,P02,P12,P012(g1)  7: S3(g2 sig)
                feat = featp.tile([128, 8 * BT], BF16, tag="feat")

                def fsl(i):
                    return feat[:, i * BT:(i + 1) * BT]
                s3 = fsl(7)
                for i in range(4):
                    rp = ps.tile([128, BT], F32, tag="route_ps")
                    nc.tensor.matmul(rp[:], sw_sb[:, i * 128:(i + 1) * 128],
                                     x[:])
                    dst = fsl(i) if i < 3 else s3
                    nc.scalar.activation(dst, rp[:], SIG,
                                         bias=sb_sb[:, i:i + 1])
                # g1 products
                nc.vector.tensor_mul(fsl(3), fsl(0), fsl(1))
                nc.vector.tensor_mul(fsl(4), fsl(0), fsl(2))
                nc.vector.tensor_mul(fsl(5), fsl(1), fsl(2))
                nc.vector.tensor_mul(fsl(6), fsl(3), fsl(2))
                # wait: fsl(3) overwritten before use as R0? no: products use
                # fsl(0..2) only, and fsl(3) (P01) written then read for P012.

                # g2: aligned copies of r1, r2 at partitions 0..21
                sc = smp.tile([G2, 2 * BT], BF16, tag="sc")
                nc.sync.dma_start(sc[:, 0:BT], s3[32:32 + G2, :])
                nc.sync.dma_start(sc[:, BT:2 * BT], s3[64:64 + G2, :])
                r0g, r1g, r2g = s3[0:G2, :], sc[:, 0:BT], sc[:, BT:2 * BT]
                # g2 products: q01,q02,q012 in scratch; q12 direct into packB
                qt = smp.tile([G2, 3 * BT], BF16, tag="qt")
                q01, q02, q012 = (qt[:, 0:BT], qt[:, BT:2 * BT],
                                  qt[:, 2 * BT:3 * BT])
                packA = smp.tile([110, BT], BF16, tag="packA")
                packB = smp.tile([44, BT], BF16, tag="packB")
                nc.vector.tensor_mul(q01, r0g, r1g)
                nc.vector.tensor_mul(q02, r0g, r2g)
                nc.vector.tensor_mul(packB[0:G2, :], r1g, r2g)      # q12
                nc.vector.tensor_mul(q012, q01, r2g)
                # pack: A=[r0,r1,r2,q01,q02], B=[q12(direct),q012]
                nc.sync.dma_start(packA[0:G2, :], r0g)
                nc.sync.dma_start(packA[G2:2 * G2, :], r1g)
                nc.sync.dma_start(packA[2 * G2:3 * G2, :], r2g)
                nc.sync.dma_start(packA[3 * G2:4 * G2, :], q01)
                nc.sync.dma_start(packA[4 * G2:5 * G2, :], q02)
                nc.sync.dma_start(packB[G2:2 * G2, :], q012)

                # final contraction: 7 g1 chunks + packA + packB
                op = pso.tile([C, BT], F32, tag="out_ps")
                for i in range(7):
                    nc.tensor.matmul(op[:], a_sb[:, i * C:(i + 1) * C],
                                     fsl(i), start=(i == 0), stop=False)
                nc.tensor.matmul(op[:], a_sb[0:110, 7 * C:8 * C], packA[:],
                                 start=False, stop=False)
                nc.tensor.matmul(op[:], a_sb[0:44, 8 * C:9 * C], packB[:],
                                 start=False, stop=True)
                ob = obp.tile([C, BT], F32, tag="ob")
                nc.vector.tensor_copy(ob[:], op[:])
                nc.sync.dma_start(out[:, j * BT:(j + 1) * BT], ob[:])
    nc.compile()
    return nc


def _get(name, builder):
    if name not in _CACHE:
        _CACHE[name] = builder()
    return _CACHE[name]


# ----------------------------------------------------------------- host math
def _layernorm(x, g, b):
    m = x.mean(-1, keepdims=True)
    v = ((x - m) ** 2).mean(-1, keepdims=True)
    return (x - m) / np.sqrt(v + LN_EPS) * g + b


def _monomial_coeffs():
    cf = np.zeros((L, 8), np.float64)
    for leaf in range(L):
        poly = np.zeros(8)
        poly[0] = 1.0
        for d in range(DEPTH):
            bit = (leaf >> d) & 1
            new = np.zeros(8)
            for S in range(8):
                if poly[S]:
                    if bit == 0:
                        new[S | (1 << d)] += poly[S]
                    else:
                        new[S] += poly[S]
                        new[S | (1 << d)] -= poly[S]
            poly = new
        cf[leaf] = poly
    return cf


def kernel(**inputs):
    f32 = lambda k: np.asarray(inputs[k], np.float32)
    X_train, X_test = f32("X_train"), f32("X_test")
    head_W2, head_b2 = np.asarray(inputs["head_W2"]), f32("head_b2")

    cores = list(range(NCORES))
    nc1 = _get("l1", _build_l1)
    nc2 = _get("l2", _build_l2)
    nc3 = _get("l3", {1: _build_l3, 2: _build_l3_v2,
                      3: _build_l3_v3}[L3_VERSION])

    # ---- L1: X_train column sums
    xtr = np.ascontiguousarray(
        X_train.reshape(NCORES, BTR_CORE, D).transpose(0, 2, 1)).astype(BFNP)
    r1 = run_bass_kernel_spmd(nc1, [{"xt": xtr[i]} for i in cores], cores)
    colsum = np.sum([r1.results[i]["s"][:, 0] for i in cores], axis=0)
    mean = (colsum / 100000.0).astype(np.float32)

    # ---- host: tiny encoder + per-class head_W1
    h = np.maximum(_layernorm(f32("enc_W1") @ mean + f32("enc_b1"),
                              f32("ln1_g"), f32("ln1_b")), 0)
    h = np.maximum(_layernorm(f32("enc_W2") @ h + f32("enc_b2"),
                              f32("ln2_g"), f32("ln2_b")), 0)
    hh = np.maximum(np.einsum('chd,d->ch', f32("head_W1"), h)
                    + f32("head_b1"), 0).astype(np.float32)   # [C, H]

    # ---- L2: used rows of head_W2, transposed + bf16, sharded by columns
    p_idx = (np.arange(T)[:, None] * PPT + USED_OFF[None, :]).ravel()
    W2u = np.empty((RTOT, H), BFNP)
    for c in range(C):
        W2u[c * RPT:(c + 1) * RPT] = head_W2[c][p_idx].astype(BFNP)
    W2T = np.zeros((H, NCORES * L2_COLS), BFNP)
    W2T[:, :RTOT] = W2u.T
    hht = np.ascontiguousarray(
        hh.astype(BFNP).T.reshape(4, 128, C))
    in2 = [{"w2t": np.ascontiguousarray(W2T[:, i * L2_COLS:(i + 1) * L2_COLS]),
            "hht": hht} for i in cores]
    r2 = run_bass_kernel_spmd(nc2, in2, cores)
    pa = np.concatenate([r2.results[i]["pr"] for i in cores], axis=1)
    cols = np.arange(RTOT)
    b2u = np.concatenate([head_b2[c][p_idx] for c in range(C)])
    pu = (pa[cols // RPT, cols] + b2u).reshape(NCT, USED)

    # ---- host: coefficient matrices
    SW = pu[:, :3 * D].reshape(NCT, 3, D)
    sbv = pu[:, 3 * D:3 * D + 3]
    leaf = pu[:, 3 * D + 3:].reshape(NCT, L, C).astype(np.float64)
    e = np.exp(leaf - leaf.max(-1, keepdims=True))
    tree_out = e / e.sum(-1, keepdims=True)
    tw = f32("tree_weights").astype(np.float64)
    w = np.exp(tw - tw.max())
    w = w / w.sum()
    wct = np.tile(w, C) / C
    M = tree_out * wct[:, None, None]                 # [NCT, L, C]
    A = np.einsum('ls,nlk->nsk', _monomial_coeffs(), M).astype(np.float32)
    const = A[:, 0, :].sum(0).astype(np.float32)      # [C]

    if L3_VERSION in (1, 3):
        sw_d = np.zeros((N_RCHUNK, 128, 128), BFNP)
        sb_d = np.zeros((128, N_RCHUNK), np.float32)
        a_d = np.zeros((N_FCHUNK, 128, C), BFNP)
        SMASK = {0: 0b001, 1: 0b010, 2: 0b100}
        PMASK = [0b011, 0b101, 0b110, 0b111]
        for g in range(2):
            ct = slice(g * GSZ, (g + 1) * GSZ)
            for d in range(3):
                ci = 3 * g + d
                sw_d[ci, :, :GSZ] = SW[ct, d, :].T.astype(BFNP)
                sb_d[:GSZ, ci] = sbv[ct, d]
                a_d[ci, :GSZ, :] = A[ct, SMASK[d], :].astype(BFNP)
            for q in range(4):
                a_d[N_RCHUNK + 4 * g + q, :GSZ, :] = \
                    A[ct, PMASK[q], :].astype(BFNP)
    else:
        G1, G2 = 128, 22
        g1, g2 = slice(0, G1), slice(G1, NCT)
        sw_d = np.zeros((4, 128, 128), BFNP)
        sb_d = np.zeros((128, 4), np.float32)
        a_d = np.zeros((9, 128, C), BFNP)
        for d in range(3):
            sw_d[d, :, :G1] = SW[g1, d, :].T.astype(BFNP)
            sb_d[:G1, d] = sbv[g1, d]
            # packed g2 route chunk: d at columns 32*d .. 32*d+22
            sw_d[3, :, 32 * d:32 * d + G2] = SW[g2, d, :].T.astype(BFNP)
            sb_d[32 * d:32 * d + G2, 3] = sbv[g2, d]
        # final chunk order: R0,R1,R2,P01,P02,P12,P012 (g1), packA, packB
        for i, S in enumerate([0b001, 0b010, 0b100, 0b011, 0b101, 0b110,
                               0b111]):
            a_d[i, :G1, :] = A[g1, S, :].astype(BFNP)
        for q, S in enumerate([0b001, 0b010, 0b100, 0b011, 0b101]):
            a_d[7, q * G2:(q + 1) * G2, :] = A[g2, S, :].astype(BFNP)
        for q, S in enumerate([0b110, 0b111]):
            a_d[8, q * G2:(q + 1) * G2, :] = A[g2, S, :].astype(BFNP)

    # ---- L3: routing over X_test shards
    xte = np.zeros((NCORES, 128, BPAD), BFNP)
    xte[:, :, :BTR_CORE] = X_test.reshape(
        NCORES, BTR_CORE, D).transpose(0, 2, 1).astype(BFNP)
    in3 = [{"xt": np.ascontiguousarray(xte[i]), "sw": sw_d, "sbias": sb_d,
            "am": a_d} for i in cores]
    r3 = run_bass_kernel_spmd(nc3, in3, cores)
    if L3_VERSION == 3:
        parts = [sum(r3.results[i]["out"][32 * s:32 * s + C, :BTR_CORE]
                     for s in range(3)) for i in cores]
        outT = np.concatenate(parts, axis=1)
    else:
        outT = np.concatenate(
            [r3.results[i]["out"][:, :BTR_CORE] for i in cores], axis=1)
    return (outT.T + const[None, :]).astype(np.float32)
